# revision 18
# baseline (speedup 1.0000x reference)
"""Trainium2 Bass kernel: channel-attention encoder (4,512,64,64), 8-core SPMD.

Sharding: 8 cores = (batch b in 0..4) x (query-half h in 0..2).  Each core
computes softmax attention for its 2048 queries over all 4096 keys of its
batch -- fully data/sequence-parallel, no collectives.

For h=1 cores the key axis of `fe` (and the packed positional tensor) is
rotated by 2048 on the host so that each core's queries are always columns
0..2048 of its own `fe` input -- softmax output is invariant under key
permutation, keeping the device graph identical across cores (SPMD).

Device math per core (C=512, C8=64, NQ=2048, NM=4096):
  k   [128,2048]  = WkT.T @ fe + bk + pos   (even key-blocks on partitions
                    0:64, odd on 64:128, written directly via col
                    tile_position -- no pack DMAs)
  vT  [4096,512]  = fe.T @ WvT  in fp8 DoubleRow (fe cast to fp8 on-device,
                    WvT pre-paired fp8 on host, x16 scaling folded out via
                    gamma/16 in the vT8 cast epilogue)
  q   [64,2048]   = WqT.T @ tot + bq, duplicated to hi partitions

  Softmax pipeline, 4 query-chunks of 512, two-chunk-lookahead mesh so the
  PE never idles at chunk boundaries:
    chunk n emits, per jj (j-pair):  exf(n,jj) [DVE], S(n+1,jj) [PE, reads
    exb written a chunk earlier], energy+exp(n+2,jj) [PE+ACT], AV(n,jj)
    [PE fp8 DoubleRow].
  1/S via int-magic seed + one Newton step on [1,512] (DVE smalls), then a
  ones-outer-product MM broadcast and one ACT copy (x-1) -> scale2 bf16.
  exf = exb * scale2 in one [128,2048] DVE op per j-pair -> exact softmax
  weights in fp8, so AV needs no epilogue normalization.
  Epilogue per (chunk, cb): out = pout + gamma*bv + fe  (fe re-read from
  the bf16 fe8 tile -- no separate f32 residual input).

PSUM: pe 2 + pout 4 + S 2 = 8 banks.
"""

import os
from contextlib import ExitStack

import numpy as np

try:
    import concourse.bass as bass
except ImportError:  # container default path
    import sys

    sys.path.insert(0, "/opt/trn_rl_repo")
    import concourse.bass as bass

import concourse.mybir as mybir
import concourse.tile as tile
import ml_dtypes
from concourse import bacc
from concourse.bass_utils import run_bass_kernel_spmd

B, C, HH, WW = 4, 512, 64, 64
C8, HW, NQ = 64, 4096, 2048
NCORES = 8
SHIFT = 60.0  # global softmax shift; energies measured in [-89, 97]
RECIP_MAGIC = float(0x7EF127EA)  # float32 reciprocal bit-trick constant

F32 = mybir.dt.float32
BF16 = mybir.dt.bfloat16
F32R = mybir.dt.float32r
FP8 = mybir.dt.float8e4
I32 = mybir.dt.int32
AF = mybir.ActivationFunctionType
ALU = mybir.AluOpType
DR = mybir.MatmulPerfMode.DoubleRow


def build_bass():
    nc = bacc.Bacc()

    fe8_d = nc.declare_dram_parameter("fe8", [C, HW], BF16, isOutput=False)
    tot_d = nc.declare_dram_parameter("tot", [C, NQ], BF16, isOutput=False)
    wqT_d = nc.declare_dram_parameter("wqT", [C, C8], BF16, isOutput=False)
    wkT_d = nc.declare_dram_parameter("wkT", [C, C8], BF16, isOutput=False)
    wvT8_d = nc.declare_dram_parameter("wvT8", [128, 2048], FP8, isOutput=False)
    pos_d = nc.declare_dram_parameter("pos", [128, 2048], BF16, isOutput=False)
    smalls_d = nc.declare_dram_parameter("smalls", [128, 8], F32, isOutput=False)
    onesr_d = nc.declare_dram_parameter("onesr", [128, 129], F32R, isOutput=False)
    out_d = nc.declare_dram_parameter("out", [C, NQ], BF16, isOutput=True)

    with ExitStack() as ctx:
        tc = ctx.enter_context(tile.TileContext(nc))
        consts = ctx.enter_context(tc.tile_pool(name="consts", bufs=1))
        big = ctx.enter_context(tc.tile_pool(name="big", bufs=1))
        stage_f8 = ctx.enter_context(tc.tile_pool(name="stage_f8", bufs=1))
        pe_pool = ctx.enter_context(tc.tile_pool(name="pe", bufs=3, space="PSUM"))
        pout_pool = ctx.enter_context(tc.tile_pool(name="pout", bufs=4, space="PSUM"))
        s_pool = ctx.enter_context(tc.tile_pool(name="spsum", bufs=1, space="PSUM"))

        # ---- preamble loads, split across sync + scalar DMA queues; fe8
        # pieces cc-major so k/v projections start on the first column block
        wkT = consts.tile([128, 4 * C8], BF16, tag="wkT", name="wkT")
        wqT = consts.tile([128, 4 * C8], BF16, tag="wqT", name="wqT")
        wvT8 = consts.tile([128, 2048], FP8, tag="wvT8", name="wvT8")
        pos = consts.tile([128, 2048], BF16, tag="pos", name="pos")
        smalls = consts.tile([128, 8], F32, tag="smalls", name="smalls")
        onesr = consts.tile([128, 129], F32R, tag="onesr", name="onesr")
        fe8 = big.tile([128, 4 * HW], BF16, tag="fe8", name="fe8")
        fe8f8 = stage_f8.tile([128, 4 * HW], FP8, tag="fe8f8", name="fe8f8")
        tot_sb = stage_f8.tile([128, 4 * NQ], BF16, tag="tot_sb", name="tot_sb")

        # three DMA queues (sync/scalar/gpsimd) so descriptor issue and
        # transfer parallelize; fe pieces cc-major so each cc block
        # completes with at most two transfers per queue
        fe_q = [nc.sync, nc.scalar, nc.gpsimd, nc.sync]
        nc.sync.dma_start(smalls[:], smalls_d[:, :])
        nc.scalar.dma_start(onesr[:], onesr_d[:, :])
        nc.gpsimd.dma_start(pos[:], pos_d[:, :])
        nc.gpsimd.dma_start(wvT8[:], wvT8_d[:, :])
        for kc in range(4):
            fe_q[kc].dma_start(
                wkT[:, kc * C8 : (kc + 1) * C8], wkT_d[kc * 128 : (kc + 1) * 128, :]
            )

        def fe_piece(cc, kc):
            fe_q[kc].dma_start(
                fe8[:, kc * HW + cc * 1024 : kc * HW + (cc + 1) * 1024],
                fe8_d[kc * 128 : (kc + 1) * 128, cc * 1024 : (cc + 1) * 1024],
            )

        for kc in range(4):
            fe_piece(0, kc)
        for kc in range(4):
            fe_q[kc].dma_start(
                tot_sb[:, kc * NQ : (kc + 1) * NQ], tot_d[kc * 128 : (kc + 1) * 128, :]
            )
        for kc in range(4):
            fe_q[kc].dma_start(
                wqT[:, kc * C8 : (kc + 1) * C8], wqT_d[kc * 128 : (kc + 1) * 128, :]
            )
        for cc in range(1, 4):
            for kc in range(4):
                fe_piece(cc, kc)

        bq_sb = smalls[:, 0:1]
        bk_sb = smalls[:, 1:2]
        bv4_sb = smalls[:, 2:6]
        g_sb = smalls[:, 6:7]
        g16_sb = smalls[:, 7:8]
        ones1 = onesr[0:1, 1:129]

        gb_sb = consts.tile([128, 4], F32, tag="gb", name="gb_sb")
        nc.vector.tensor_scalar_mul(gb_sb[:], bv4_sb, g_sb)
        negshift = consts.tile([128, 1], F32, tag="negshift", name="negshift")
        nc.vector.memset(negshift[:], -SHIFT)
        negones = consts.tile([1, 512], I32, tag="negones", name="negones")
        nc.vector.memset(negones[:], -1)
        ones_b = consts.tile([128, 1], BF16, tag="ones_b", name="ones_b")
        nc.vector.tensor_copy(ones_b[:], onesr[:, 0:1].bitcast(F32))

        q_sb = big.tile([128, NQ], BF16, tag="q_sb", name="q_sb")
        kpos = big.tile([128, 16 * 128], BF16, tag="kpos", name="kpos")
        vT8 = big.tile([128, 32 * C], FP8, tag="vT8", name="vT8")

        fe4 = fe8[:].rearrange("p (kc m) -> p kc m", kc=4)
        f84 = fe8f8[:].rearrange("p (kc m) -> p kc m", kc=4)
        wv4 = wvT8[:].rearrange("p (k t c) -> p k t c", k=2, t=2)

        # ---- k-proj (packed layout via col tile_position), per fe8 cc-block;
        # chunk-0 energies and fp8 casts of fe interleave as each cc lands
        def k_proj(cc):
            pk = pout_pool.tile([128, 512], F32, tag="pout", name="pk")
            for eo in range(2):
                for kc in range(4):
                    mv = (
                        fe8[:, kc * HW + cc * 1024 : kc * HW + (cc + 1) * 1024]
                        .rearrange("p (b2 eo r) -> p eo b2 r", b2=4, eo=2)[:, eo]
                    )
                    nc.tensor.matmul(
                        pk[eo * 64 : (eo + 1) * 64, :],
                        wkT[:, kc * C8 : (kc + 1) * C8],
                        mv,
                        start=(kc == 0),
                        stop=(kc == 3),
                    )
            sl = kpos[:, 4 * cc * 128 : 4 * cc * 128 + 512]
            nc.vector.scalar_tensor_tensor(
                sl, pk[:], bk_sb, pos[:, 4 * cc * 128 : 4 * cc * 128 + 512], ALU.add, ALU.add
            )

        # ---- q = WqT.T @ tot + bq, computed into both partition halves
        # (hi half via col tile_position -- no SBUF-to-SBUF duplication DMA)
        def q_proj():
            for nch in range(4):
                pq = pout_pool.tile([128, 512], F32, tag="pout", name="pq")
                for hi in range(2):
                    for kc in range(4):
                        nc.tensor.matmul(
                            pq[hi * C8 : (hi + 1) * C8, :],
                            wqT[:, kc * C8 : (kc + 1) * C8],
                            tot_sb[:, kc * NQ + nch * 512 : kc * NQ + (nch + 1) * 512],
                            start=(kc == 0),
                            stop=(kc == 3),
                        )
                nc.scalar.activation(
                    q_sb[:, nch * 512 : (nch + 1) * 512], pq[:], AF.Identity, bias=bq_sb
                )

        work = ctx.enter_context(tc.tile_pool(name="work", bufs=2))
        exbs = ctx.enter_context(tc.tile_pool(name="exbs", bufs=16))
        exfs = ctx.enter_context(tc.tile_pool(name="exfs", bufs=10))
        exsums = ctx.enter_context(tc.tile_pool(name="exsums", bufs=4))

        NCH = 4
        st = {}  # (kind, nch, jj) / (kind, nch) -> tile

        def e_block(nch, jj):
            """Energies + exp for j-pair (2jj, 2jj+1) -> exb2 [128,2048] bf16."""
            q_lo = q_sb[0:C8, nch * 512 : (nch + 1) * 512]
            q_hi = q_sb[C8:128, nch * 512 : (nch + 1) * 512]
            exb2 = exbs.tile([128, 2048], BF16, tag="exb", name=f"exb{nch}_{jj}")
            st[("exb", nch, jj)] = exb2
            for jl in range(2):
                j = 2 * jj + jl
                for half in range(2):
                    pe = pe_pool.tile([128, 512], F32, tag="pe", name="pe")
                    nc.tensor.matmul(
                        pe[:],
                        kpos[half * C8 : (half + 1) * C8, j * 128 : (j + 1) * 128],
                        q_lo if half == 0 else q_hi,
                        start=True,
                        stop=True,
                        tile_position=(half * C8, 0),
                    )
                    nc.scalar.activation(
                        exb2[:, jl * 1024 + half * 512 : jl * 1024 + (half + 1) * 512],
                        pe[:],
                        AF.Exp,
                        bias=negshift[:, 0:1],
                    )

        def s_block(nch, jj):
            """Pre-scale key-sums: DVE pre-adds the j-pair, S accumulates 16 MMs."""
            if jj == 0:
                st[("S", nch)] = s_pool.tile([1, 512], F32, tag="spsum", name=f"S{nch}")
            S = st[("S", nch)]
            exb2 = st[("exb", nch, jj)]
            exsum = exsums.tile([128, 1024], BF16, tag="exsum", name=f"exsum{nch}_{jj}")
            nc.vector.tensor_tensor(exsum[:], exb2[:, 0:1024], exb2[:, 1024:2048], ALU.add)
            for s in range(2):
                nc.tensor.matmul(
                    S[:],
                    ones_b[:],
                    exsum[:, s * 512 : (s + 1) * 512],
                    start=(jj == 0 and s == 0),
                    stop=(jj == 7 and s == 1),
                    skip_group_check=True,
                )

        def magic_chain(nch):
            """scale2 = bcast(1/S) bf16 via int-magic seed + one Newton step."""
            S = st[("S", nch)]
            r0i = work.tile([1, 512], I32, tag="r0i", name="r0i")
            with nc.allow_low_precision(reason="int32 bit arithmetic for recip seed"):
                nc.vector.scalar_tensor_tensor(
                    r0i[:], S[:].bitcast(I32), RECIP_MAGIC, negones[:], ALU.subtract, ALU.mult
                )
                t = work.tile([1, 512], F32, tag="tmag", name="tmag")
                nc.vector.tensor_tensor(t[:], S[:], r0i[:].bitcast(F32), ALU.mult)
                rT = work.tile([1, 512], F32R, tag="rT", name="rT")
                # (t - 2) * r0 = -1/S after one Newton step
                nc.vector.scalar_tensor_tensor(
                    rT[:], t[:], 2.0, r0i[:].bitcast(F32), ALU.subtract, ALU.mult
                )
                pbs = pe_pool.tile([128, 512], F32, tag="pe", name="pbs")
                nc.tensor.matmul(pbs[:], ones1, rT[:], start=True, stop=True)
            scale2 = work.tile([128, 512], BF16, tag="scale2", name="scale2")
            nc.scalar.mul(scale2[:], pbs[:], -1.0)
            st[("scale2", nch)] = scale2

        def exf_block(nch, jj):
            exb2 = st.pop(("exb", nch, jj))
            scale2 = st[("scale2", nch)]
            exf2 = exfs.tile([128, 2048], FP8, tag="exf", name=f"exf{nch}_{jj}")
            st[("exf", nch, jj)] = exf2
            e3 = exb2[:].rearrange("p (f n) -> p f n", f=4)
            x3 = exf2[:].rearrange("p (f n) -> p f n", f=4)
            s3 = scale2[:].unsqueeze(1).broadcast_to([128, 4, 512])
            nc.vector.tensor_tensor(x3, e3, s3, ALU.mult)

        def b_group(nch, jj, pouts):
            """AV DoubleRow MMs for j-pair across the 4 output channel blocks."""
            exf2 = st.pop(("exf", nch, jj))
            for jl in range(2):
                j = 2 * jj + jl
                exf_dr = exf2[:, jl * 1024 : (jl + 1) * 1024].rearrange(
                    "p (two n) -> p two n", two=2
                )
                vblk = vT8[:, j * 1024 : (j + 1) * 1024].rearrange(
                    "p (two c) -> p two c", two=2
                )
                for cb in range(4):
                    nc.tensor.matmul(
                        pouts[cb][:],
                        vblk[:, :, cb * 128 : (cb + 1) * 128],
                        exf_dr,
                        start=(j == 0),
                        stop=(j == 15),
                        perf_mode=DR,
                        skip_group_check=True,
                    )

        def epi_one(nch, cb, pout):
            outst = work.tile([128, 512], BF16, tag="outst", name="outst")
            fe_res = fe8[:, cb * HW + nch * 512 : cb * HW + (nch + 1) * 512]
            nc.vector.scalar_tensor_tensor(
                outst[:], pout[:], gb_sb[:, cb : cb + 1], fe_res, ALU.add, ALU.add
            )
            eng = nc.sync if cb % 2 == 0 else nc.scalar
            eng.dma_start(
                out_d[cb * 128 : (cb + 1) * 128, nch * 512 : (nch + 1) * 512], outst[:]
            )

        # ---- ramp: per-cc [k-proj, fp8 casts, v-proj] keeps the PE dense
        # while fe streams in; q after cc1 (tot has landed); chunk-0 energies
        # spliced in from cc2 so exps(0) overlap the rest of the preamble
        def kv_cc(cc):
            k_proj(cc)
            for kc in range(4):
                nc.vector.tensor_copy(
                    fe8f8[:, kc * HW + cc * 1024 : kc * HW + (cc + 1) * 1024],
                    fe8[:, kc * HW + cc * 1024 : kc * HW + (cc + 1) * 1024],
                )
            for mb in range(8 * cc, 8 * cc + 8):
                pv = pout_pool.tile([128, 512], F32, tag="pout", name="pv")
                for kc2 in range(2):
                    stat = f84[:, 2 * kc2 : 2 * kc2 + 2, mb * 128 : (mb + 1) * 128]
                    nc.tensor.matmul(
                        pv[:],
                        stat,
                        wv4[:, kc2],
                        start=(kc2 == 0),
                        stop=(kc2 == 1),
                        perf_mode=DR,
                    )
                # fp8 cast with gamma/16 folded in, split across DVE and ACT
                if mb % 2 == 0:
                    nc.vector.tensor_scalar_mul(vT8[:, mb * C : (mb + 1) * C], pv[:], g16_sb)
                else:
                    nc.scalar.mul(vT8[:, mb * C : (mb + 1) * C], pv[:], g16_sb)

        kv_cc(0)
        q_proj()
        kv_cc(1)
        e_block(0, 0)
        e_block(0, 1)
        kv_cc(2)
        e_block(0, 2)
        e_block(0, 3)
        s_block(0, 0)
        s_block(0, 1)
        kv_cc(3)
        e_block(0, 4)
        e_block(0, 5)
        s_block(0, 2)
        s_block(0, 3)
        e_block(0, 6)
        e_block(0, 7)
        for jj in range(4, 8):
            s_block(0, jj)
        magic_chain(0)

        # ---- mesh: chunk n emits exf(n) | energy(n+1) | S(n+1, jj-2) | AV(n)
        for nch in range(NCH):
            pouts = [
                pout_pool.tile([128, 512], F32, tag="pout", name=f"pout{nch}_{cb}")
                for cb in range(4)
            ]
            for jj in range(8):
                exf_block(nch, jj)
                if nch + 1 < NCH:
                    e_block(nch + 1, jj)
                    if jj >= 2:
                        s_block(nch + 1, jj - 2)
                b_group(nch, jj, pouts)
            if nch + 1 < NCH:
                s_block(nch + 1, 6)
                s_block(nch + 1, 7)
                magic_chain(nch + 1)
            for cb in range(4):
                epi_one(nch, cb, pouts[cb])

    nc.compile()
    return nc


def make_in_maps(final_encoded, total, Wq, bq, Wk, bk, Wv, bv, height_tensor, width_tensor, gamma):
    f32 = np.float32
    bf16 = ml_dtypes.bfloat16
    fp8 = ml_dtypes.float8_e4m3
    fe = np.ascontiguousarray(final_encoded, f32).reshape(B, C, HW)
    tot = np.ascontiguousarray(total, f32).reshape(B, C, HW)
    wqT = np.ascontiguousarray(np.asarray(Wq, f32).T.astype(bf16))
    wkT = np.ascontiguousarray(np.asarray(Wk, f32).T.astype(bf16))
    # wvT8: [c, ch] -> pairs (kc2, t) along c, x16 into fp8 normal range
    vt = np.asarray(Wv, f32).T * 16.0
    wvT8 = np.ascontiguousarray(
        vt.reshape(2, 2, 128, C).transpose(2, 0, 1, 3).reshape(128, 2048).astype(fp8)
    )
    hb = np.asarray(height_tensor, f32).reshape(C8, HH)
    wd = np.asarray(width_tensor, f32).reshape(C8, WW)
    pos_full = (hb[:, :, None] + wd[:, None, :]).reshape(C8, HW)

    def pack_pos(pf):
        blocks = pf.reshape(C8, 32, 128)
        pp = np.concatenate([blocks[:, 0::2], blocks[:, 1::2]], axis=0)
        return np.ascontiguousarray(pp.reshape(128, 2048).astype(bf16))

    pos0 = pack_pos(pos_full)
    pos1 = pack_pos(np.roll(pos_full, -NQ, axis=1))

    g = float(np.asarray(gamma, f32).reshape(-1)[0])
    s = np.zeros((128, 8), f32)
    s[0:C8, 0] = np.asarray(bq, f32).reshape(-1)
    s[C8:128, 0] = np.asarray(bq, f32).reshape(-1)
    s[0:C8, 1] = np.asarray(bk, f32).reshape(-1)
    s[C8:128, 1] = np.asarray(bk, f32).reshape(-1)
    s[:, 2:6] = np.asarray(bv, f32).reshape(4, 128).T
    s[:, 6] = g
    s[:, 7] = g / 16.0
    ones_arr = np.ones((128, 129), f32)

    in_maps = []
    for core in range(NCORES):
        b, h = core // 2, core % 2
        fe_c = fe[b] if h == 0 else np.ascontiguousarray(np.roll(fe[b], -NQ, axis=1))
        in_maps.append(
            {
                "fe8": np.ascontiguousarray(fe_c.astype(bf16)),
                "tot": np.ascontiguousarray(tot[b][:, h * NQ : (h + 1) * NQ].astype(bf16)),
                "wqT": wqT,
                "wkT": wkT,
                "wvT8": wvT8,
                "pos": pos0 if h == 0 else pos1,
                "smalls": s,
                "onesr": ones_arr,
            }
        )
    return in_maps


def unshard(results):
    out = np.empty((B, C, HW), np.float32)
    for core in range(NCORES):
        b, h = core // 2, core % 2
        out[b][:, h * NQ : (h + 1) * NQ] = results[core]["out"]
    return out.reshape(B, C, HH, WW)


_NC = None


def get_nc():
    global _NC
    if _NC is None:
        _NC = build_bass()
    return _NC


def run_cores(in_maps, **kwargs):
    return run_bass_kernel_spmd(get_nc(), in_maps, core_ids=list(range(NCORES)), **kwargs)


def kernel(**inputs):
    in_maps = make_in_maps(**inputs)
    res = run_cores(in_maps)
    return unshard(res.results)


# revision 19
# speedup vs baseline: 1.0391x; 1.0391x over previous
"""Trainium2 Bass kernel: channel-attention encoder (4,512,64,64), 8-core SPMD.

Sharding: 8 cores = (batch b in 0..4) x (query-half h in 0..2).  Each core
computes softmax attention for its 2048 queries over all 4096 keys of its
batch -- fully data/sequence-parallel, no collectives.

For h=1 cores the key axis of `fe` (and the packed positional tensor) is
rotated by 2048 on the host so that each core's queries are always columns
0..2048 of its own `fe` input -- softmax output is invariant under key
permutation, keeping the device graph identical across cores (SPMD).

Device math per core (C=512, C8=64, NQ=2048, NM=4096):
  k   [128,2048]  = WkT.T @ fe + bk + pos   (even key-blocks on partitions
                    0:64, odd on 64:128, written directly via col
                    tile_position -- no pack DMAs)
  vT  [4096,512]  = fe.T @ WvT  in fp8 DoubleRow (fe cast to fp8 on-device,
                    WvT pre-paired fp8 on host, x16 scaling folded out via
                    gamma/16 in the vT8 cast epilogue)
  q   [64,2048]   = WqT.T @ tot + bq, duplicated to hi partitions

  Softmax pipeline, 4 query-chunks of 512, two-chunk-lookahead mesh so the
  PE never idles at chunk boundaries:
    chunk n emits, per jj (j-pair):  exf(n,jj) [DVE], S(n+1,jj) [PE, reads
    exb written a chunk earlier], energy+exp(n+2,jj) [PE+ACT], AV(n,jj)
    [PE fp8 DoubleRow].
  1/S via int-magic seed + one Newton step on [1,512] (DVE smalls), then a
  ones-outer-product MM broadcast and one ACT copy (x-1) -> scale2 bf16.
  exf = exb * scale2 in one [128,2048] DVE op per j-pair -> exact softmax
  weights in fp8, so AV needs no epilogue normalization.
  Epilogue per (chunk, cb): out = pout + gamma*bv + fe  (fe re-read from
  the bf16 fe8 tile -- no separate f32 residual input).

PSUM: pe 2 + pout 4 + S 2 = 8 banks.
"""

import os
from contextlib import ExitStack

import numpy as np

try:
    import concourse.bass as bass
except ImportError:  # container default path
    import sys

    sys.path.insert(0, "/opt/trn_rl_repo")
    import concourse.bass as bass

import concourse.mybir as mybir
import concourse.tile as tile
import ml_dtypes
from concourse import bacc
from concourse.bass_utils import run_bass_kernel_spmd

B, C, HH, WW = 4, 512, 64, 64
C8, HW, NQ = 64, 4096, 2048
NCORES = 8
SHIFT = 60.0  # global softmax shift; energies measured in [-89, 97]
RECIP_MAGIC = float(0x7EF127EA)  # float32 reciprocal bit-trick constant

F32 = mybir.dt.float32
BF16 = mybir.dt.bfloat16
F32R = mybir.dt.float32r
FP8 = mybir.dt.float8e4
I32 = mybir.dt.int32
AF = mybir.ActivationFunctionType
ALU = mybir.AluOpType
DR = mybir.MatmulPerfMode.DoubleRow


def build_bass():
    nc = bacc.Bacc()

    fe8_d = nc.declare_dram_parameter("fe8", [C, HW], BF16, isOutput=False)
    tot_d = nc.declare_dram_parameter("tot", [C, NQ], BF16, isOutput=False)
    wqT_d = nc.declare_dram_parameter("wqT", [C, C8], BF16, isOutput=False)
    wkT_d = nc.declare_dram_parameter("wkT", [C, C8], BF16, isOutput=False)
    wvT8_d = nc.declare_dram_parameter("wvT8", [128, 2048], FP8, isOutput=False)
    pos_d = nc.declare_dram_parameter("pos", [128, 2048], BF16, isOutput=False)
    smalls_d = nc.declare_dram_parameter("smalls", [128, 8], F32, isOutput=False)
    onesr_d = nc.declare_dram_parameter("onesr", [128, 129], F32R, isOutput=False)
    out_d = nc.declare_dram_parameter("out", [C, NQ], BF16, isOutput=True)

    with ExitStack() as ctx:
        tc = ctx.enter_context(tile.TileContext(nc))
        consts = ctx.enter_context(tc.tile_pool(name="consts", bufs=1))
        big = ctx.enter_context(tc.tile_pool(name="big", bufs=1))
        stage_f8 = ctx.enter_context(tc.tile_pool(name="stage_f8", bufs=1))
        pe_pool = ctx.enter_context(tc.tile_pool(name="pe", bufs=3, space="PSUM"))
        pout_pool = ctx.enter_context(tc.tile_pool(name="pout", bufs=4, space="PSUM"))
        s_pool = ctx.enter_context(tc.tile_pool(name="spsum", bufs=1, space="PSUM"))

        # ---- preamble loads, split across sync + scalar DMA queues; fe8
        # pieces cc-major so k/v projections start on the first column block
        wkT = consts.tile([128, 4 * C8], BF16, tag="wkT", name="wkT")
        wqT = consts.tile([128, 4 * C8], BF16, tag="wqT", name="wqT")
        wvT8 = consts.tile([128, 2048], FP8, tag="wvT8", name="wvT8")
        pos = consts.tile([128, 2048], BF16, tag="pos", name="pos")
        smalls = consts.tile([128, 8], F32, tag="smalls", name="smalls")
        onesr = consts.tile([128, 129], F32R, tag="onesr", name="onesr")
        fe8 = big.tile([128, 4 * HW], BF16, tag="fe8", name="fe8")
        fe8f8 = stage_f8.tile([128, 4 * HW], FP8, tag="fe8f8", name="fe8f8")
        tot_sb = stage_f8.tile([128, 4 * NQ], BF16, tag="tot_sb", name="tot_sb")

        # three DMA queues (sync/scalar/gpsimd) so descriptor issue and
        # transfer parallelize; fe pieces cc-major so each cc block
        # completes with at most two transfers per queue
        fe_q = [nc.sync, nc.scalar, nc.gpsimd, nc.sync]
        nc.sync.dma_start(smalls[:], smalls_d[:, :])
        nc.scalar.dma_start(onesr[:], onesr_d[:, :])
        nc.gpsimd.dma_start(pos[:], pos_d[:, :])
        nc.gpsimd.dma_start(wvT8[:], wvT8_d[:, :])
        for kc in range(4):
            fe_q[kc].dma_start(
                wkT[:, kc * C8 : (kc + 1) * C8], wkT_d[kc * 128 : (kc + 1) * 128, :]
            )

        def fe_piece(cc, kc):
            fe_q[kc].dma_start(
                fe8[:, kc * HW + cc * 1024 : kc * HW + (cc + 1) * 1024],
                fe8_d[kc * 128 : (kc + 1) * 128, cc * 1024 : (cc + 1) * 1024],
            )

        for kc in range(4):
            fe_piece(0, kc)
        for kc in range(4):
            fe_q[kc].dma_start(
                tot_sb[:, kc * NQ : (kc + 1) * NQ], tot_d[kc * 128 : (kc + 1) * 128, :]
            )
        for kc in range(4):
            fe_q[kc].dma_start(
                wqT[:, kc * C8 : (kc + 1) * C8], wqT_d[kc * 128 : (kc + 1) * 128, :]
            )
        for cc in range(1, 4):
            for kc in range(4):
                fe_piece(cc, kc)

        bq_sb = smalls[:, 0:1]
        bk_sb = smalls[:, 1:2]
        bv4_sb = smalls[:, 2:6]
        g_sb = smalls[:, 6:7]
        g16_sb = smalls[:, 7:8]
        ones1 = onesr[0:1, 1:129]

        gb_sb = consts.tile([128, 4], F32, tag="gb", name="gb_sb")
        nc.vector.tensor_scalar_mul(gb_sb[:], bv4_sb, g_sb)
        negshift = consts.tile([128, 1], F32, tag="negshift", name="negshift")
        nc.vector.memset(negshift[:], -SHIFT)
        negones = consts.tile([1, 512], I32, tag="negones", name="negones")
        nc.vector.memset(negones[:], -1)
        ones_b = consts.tile([128, 1], BF16, tag="ones_b", name="ones_b")
        nc.vector.tensor_copy(ones_b[:], onesr[:, 0:1].bitcast(F32))

        # ~40 dummy matmuls on a memset tile keep the PE HAM-warm and the
        # queue primed while the first fe/tot DMA pieces land
        dummy_sb = consts.tile([128, 128], BF16, tag="dummy", name="dummy_sb")
        nc.vector.memset(dummy_sb[:], 1.0)
        pdump = pout_pool.tile([128, 512], F32, tag="pout", name="pdump")
        for _ in range(40):
            nc.tensor.matmul(pdump[:, 0:128], dummy_sb[:], dummy_sb[:], start=True, stop=True)

        q_sb = big.tile([128, NQ], BF16, tag="q_sb", name="q_sb")
        kpos = big.tile([128, 16 * 128], BF16, tag="kpos", name="kpos")
        vT8 = big.tile([128, 32 * C], FP8, tag="vT8", name="vT8")

        fe4 = fe8[:].rearrange("p (kc m) -> p kc m", kc=4)
        f84 = fe8f8[:].rearrange("p (kc m) -> p kc m", kc=4)
        wv4 = wvT8[:].rearrange("p (k t c) -> p k t c", k=2, t=2)

        # ---- k-proj (packed layout via col tile_position), per fe8 cc-block;
        # chunk-0 energies and fp8 casts of fe interleave as each cc lands
        def k_proj(cc):
            pk = pout_pool.tile([128, 512], F32, tag="pout", name="pk")
            for eo in range(2):
                for kc in range(4):
                    mv = (
                        fe8[:, kc * HW + cc * 1024 : kc * HW + (cc + 1) * 1024]
                        .rearrange("p (b2 eo r) -> p eo b2 r", b2=4, eo=2)[:, eo]
                    )
                    nc.tensor.matmul(
                        pk[eo * 64 : (eo + 1) * 64, :],
                        wkT[:, kc * C8 : (kc + 1) * C8],
                        mv,
                        start=(kc == 0),
                        stop=(kc == 3),
                    )
            sl = kpos[:, 4 * cc * 128 : 4 * cc * 128 + 512]
            nc.vector.scalar_tensor_tensor(
                sl, pk[:], bk_sb, pos[:, 4 * cc * 128 : 4 * cc * 128 + 512], ALU.add, ALU.add
            )

        # ---- q = WqT.T @ tot + bq, computed into both partition halves
        # (hi half via col tile_position -- no SBUF-to-SBUF duplication DMA)
        def q_proj():
            for nch in range(4):
                pq = pout_pool.tile([C8, 512], F32, tag="pout", name="pq")
                for kc in range(4):
                    nc.tensor.matmul(
                        pq[:],
                        wqT[:, kc * C8 : (kc + 1) * C8],
                        tot_sb[:, kc * NQ + nch * 512 : kc * NQ + (nch + 1) * 512],
                        start=(kc == 0),
                        stop=(kc == 3),
                    )
                lo = q_sb[0:C8, nch * 512 : (nch + 1) * 512]
                nc.scalar.activation(lo, pq[:], AF.Identity, bias=bq_sb[0:C8, 0:1])
                nc.gpsimd.dma_start(q_sb[C8:128, nch * 512 : (nch + 1) * 512], lo)

        work = ctx.enter_context(tc.tile_pool(name="work", bufs=2))
        exbs = ctx.enter_context(tc.tile_pool(name="exbs", bufs=16))
        exfs = ctx.enter_context(tc.tile_pool(name="exfs", bufs=10))
        exsums = ctx.enter_context(tc.tile_pool(name="exsums", bufs=4))

        NCH = 4
        st = {}  # (kind, nch, jj) / (kind, nch) -> tile

        def e_block(nch, jj):
            """Energies + exp for j-pair (2jj, 2jj+1) -> exb2 [128,2048] bf16."""
            q_lo = q_sb[0:C8, nch * 512 : (nch + 1) * 512]
            q_hi = q_sb[C8:128, nch * 512 : (nch + 1) * 512]
            exb2 = exbs.tile([128, 2048], BF16, tag="exb", name=f"exb{nch}_{jj}")
            st[("exb", nch, jj)] = exb2
            for jl in range(2):
                j = 2 * jj + jl
                for half in range(2):
                    pe = pe_pool.tile([128, 512], F32, tag="pe", name="pe")
                    nc.tensor.matmul(
                        pe[:],
                        kpos[half * C8 : (half + 1) * C8, j * 128 : (j + 1) * 128],
                        q_lo if half == 0 else q_hi,
                        start=True,
                        stop=True,
                        tile_position=(half * C8, 0),
                    )
                    nc.scalar.activation(
                        exb2[:, jl * 1024 + half * 512 : jl * 1024 + (half + 1) * 512],
                        pe[:],
                        AF.Exp,
                        bias=negshift[:, 0:1],
                    )
            exsum = exsums.tile([128, 1024], BF16, tag="exsum", name=f"exsum{nch}_{jj}")
            nc.vector.tensor_tensor(exsum[:], exb2[:, 0:1024], exb2[:, 1024:2048], ALU.add)
            st[("exsum", nch, jj)] = exsum

        def s_block(nch, jj):
            """Pre-scale key-sums: S accumulates 16 MMs over the pre-added pairs."""
            if jj == 0:
                st[("S", nch)] = s_pool.tile([1, 512], F32, tag="spsum", name=f"S{nch}")
            S = st[("S", nch)]
            exsum = st.pop(("exsum", nch, jj))
            for s in range(2):
                nc.tensor.matmul(
                    S[:],
                    ones_b[:],
                    exsum[:, s * 512 : (s + 1) * 512],
                    start=(jj == 0 and s == 0),
                    stop=(jj == 7 and s == 1),
                    skip_group_check=True,
                )

        def magic_chain(nch):
            """scale2 = bcast(1/S) bf16 via int-magic seed + one Newton step."""
            S = st[("S", nch)]
            r0i = work.tile([1, 512], I32, tag="r0i", name="r0i")
            with nc.allow_low_precision(reason="int32 bit arithmetic for recip seed"):
                nc.vector.scalar_tensor_tensor(
                    r0i[:], S[:].bitcast(I32), RECIP_MAGIC, negones[:], ALU.subtract, ALU.mult
                )
                t = work.tile([1, 512], F32, tag="tmag", name="tmag")
                nc.vector.tensor_tensor(t[:], S[:], r0i[:].bitcast(F32), ALU.mult)
                rT = work.tile([1, 512], F32R, tag="rT", name="rT")
                # (t - 2) * r0 = -1/S after one Newton step
                nc.vector.scalar_tensor_tensor(
                    rT[:], t[:], 2.0, r0i[:].bitcast(F32), ALU.subtract, ALU.mult
                )
                pbs = pe_pool.tile([128, 512], F32, tag="pe", name="pbs")
                nc.tensor.matmul(pbs[:], ones1, rT[:], start=True, stop=True)
            scale2 = work.tile([128, 512], BF16, tag="scale2", name="scale2")
            nc.scalar.mul(scale2[:], pbs[:], -1.0)
            st[("scale2", nch)] = scale2

        def exf_block(nch, jj):
            exb2 = st.pop(("exb", nch, jj))
            scale2 = st[("scale2", nch)]
            exf2 = exfs.tile([128, 2048], FP8, tag="exf", name=f"exf{nch}_{jj}")
            st[("exf", nch, jj)] = exf2
            e3 = exb2[:].rearrange("p (f n) -> p f n", f=4)
            x3 = exf2[:].rearrange("p (f n) -> p f n", f=4)
            s3 = scale2[:].unsqueeze(1).broadcast_to([128, 4, 512])
            nc.vector.tensor_tensor(x3, e3, s3, ALU.mult)

        def b_group(nch, jj, pouts):
            """AV DoubleRow MMs for j-pair across the 4 output channel blocks."""
            exf2 = st.pop(("exf", nch, jj))
            for jl in range(2):
                j = 2 * jj + jl
                exf_dr = exf2[:, jl * 1024 : (jl + 1) * 1024].rearrange(
                    "p (two n) -> p two n", two=2
                )
                vblk = vT8[:, j * 1024 : (j + 1) * 1024].rearrange(
                    "p (two c) -> p two c", two=2
                )
                for cb in range(4):
                    nc.tensor.matmul(
                        pouts[cb][:],
                        vblk[:, :, cb * 128 : (cb + 1) * 128],
                        exf_dr,
                        start=(j == 0),
                        stop=(j == 15),
                        perf_mode=DR,
                        skip_group_check=True,
                    )

        def epi_one(nch, cb, pout):
            outst = work.tile([128, 512], BF16, tag="outst", name="outst")
            fe_res = fe8[:, cb * HW + nch * 512 : cb * HW + (nch + 1) * 512]
            nc.vector.scalar_tensor_tensor(
                outst[:], pout[:], gb_sb[:, cb : cb + 1], fe_res, ALU.add, ALU.add
            )
            eng = nc.sync if cb % 2 == 0 else nc.scalar
            eng.dma_start(
                out_d[cb * 128 : (cb + 1) * 128, nch * 512 : (nch + 1) * 512], outst[:]
            )

        # ---- ramp: per-cc [k-proj, fp8 casts, v-proj] keeps the PE dense
        # while fe streams in; q after cc1 (tot has landed); chunk-0 energies
        # spliced in from cc2 so exps(0) overlap the rest of the preamble
        def kv_cc(cc):
            k_proj(cc)
            for kc in range(4):
                nc.vector.tensor_copy(
                    fe8f8[:, kc * HW + cc * 1024 : kc * HW + (cc + 1) * 1024],
                    fe8[:, kc * HW + cc * 1024 : kc * HW + (cc + 1) * 1024],
                )
            for mb in range(8 * cc, 8 * cc + 8):
                pv = pout_pool.tile([128, 512], F32, tag="pout", name="pv")
                for kc2 in range(2):
                    stat = f84[:, 2 * kc2 : 2 * kc2 + 2, mb * 128 : (mb + 1) * 128]
                    nc.tensor.matmul(
                        pv[:],
                        stat,
                        wv4[:, kc2],
                        start=(kc2 == 0),
                        stop=(kc2 == 1),
                        perf_mode=DR,
                    )
                # fp8 cast with gamma/16 folded in, split across DVE and ACT
                if mb % 2 == 0:
                    nc.vector.tensor_scalar_mul(vT8[:, mb * C : (mb + 1) * C], pv[:], g16_sb)
                else:
                    nc.scalar.mul(vT8[:, mb * C : (mb + 1) * C], pv[:], g16_sb)

        kv_cc(0)
        q_proj()
        kv_cc(1)
        e_block(0, 0)
        e_block(0, 1)
        kv_cc(2)
        e_block(0, 2)
        e_block(0, 3)
        s_block(0, 0)
        s_block(0, 1)
        kv_cc(3)
        e_block(0, 4)
        e_block(0, 5)
        s_block(0, 2)
        s_block(0, 3)
        e_block(0, 6)
        e_block(0, 7)
        for jj in range(4, 8):
            s_block(0, jj)
        magic_chain(0)

        # ---- mesh: chunk n emits exf(n) | energy(n+1) | S(n+1, jj-2) | AV(n)
        for nch in range(NCH):
            pouts = [
                pout_pool.tile([128, 512], F32, tag="pout", name=f"pout{nch}_{cb}")
                for cb in range(4)
            ]
            exf_block(nch, 0)
            for jj in range(8):
                if jj + 1 < 8:
                    exf_block(nch, jj + 1)
                if nch + 1 < NCH:
                    e_block(nch + 1, jj)
                    if jj >= 2:
                        s_block(nch + 1, jj - 2)
                b_group(nch, jj, pouts)
            if nch + 1 < NCH:
                s_block(nch + 1, 6)
                s_block(nch + 1, 7)
                magic_chain(nch + 1)
            for cb in range(4):
                epi_one(nch, cb, pouts[cb])

    nc.compile()
    return nc


def make_in_maps(final_encoded, total, Wq, bq, Wk, bk, Wv, bv, height_tensor, width_tensor, gamma):
    f32 = np.float32
    bf16 = ml_dtypes.bfloat16
    fp8 = ml_dtypes.float8_e4m3
    fe = np.ascontiguousarray(final_encoded, f32).reshape(B, C, HW)
    tot = np.ascontiguousarray(total, f32).reshape(B, C, HW)
    wqT = np.ascontiguousarray(np.asarray(Wq, f32).T.astype(bf16))
    wkT = np.ascontiguousarray(np.asarray(Wk, f32).T.astype(bf16))
    # wvT8: [c, ch] -> pairs (kc2, t) along c, x16 into fp8 normal range
    vt = np.asarray(Wv, f32).T * 16.0
    wvT8 = np.ascontiguousarray(
        vt.reshape(2, 2, 128, C).transpose(2, 0, 1, 3).reshape(128, 2048).astype(fp8)
    )
    hb = np.asarray(height_tensor, f32).reshape(C8, HH)
    wd = np.asarray(width_tensor, f32).reshape(C8, WW)
    pos_full = (hb[:, :, None] + wd[:, None, :]).reshape(C8, HW)

    def pack_pos(pf):
        blocks = pf.reshape(C8, 32, 128)
        pp = np.concatenate([blocks[:, 0::2], blocks[:, 1::2]], axis=0)
        return np.ascontiguousarray(pp.reshape(128, 2048).astype(bf16))

    pos0 = pack_pos(pos_full)
    pos1 = pack_pos(np.roll(pos_full, -NQ, axis=1))

    g = float(np.asarray(gamma, f32).reshape(-1)[0])
    s = np.zeros((128, 8), f32)
    s[0:C8, 0] = np.asarray(bq, f32).reshape(-1)
    s[C8:128, 0] = np.asarray(bq, f32).reshape(-1)
    s[0:C8, 1] = np.asarray(bk, f32).reshape(-1)
    s[C8:128, 1] = np.asarray(bk, f32).reshape(-1)
    s[:, 2:6] = np.asarray(bv, f32).reshape(4, 128).T
    s[:, 6] = g
    s[:, 7] = g / 16.0
    ones_arr = np.ones((128, 129), f32)

    in_maps = []
    for core in range(NCORES):
        b, h = core // 2, core % 2
        fe_c = fe[b] if h == 0 else np.ascontiguousarray(np.roll(fe[b], -NQ, axis=1))
        in_maps.append(
            {
                "fe8": np.ascontiguousarray(fe_c.astype(bf16)),
                "tot": np.ascontiguousarray(tot[b][:, h * NQ : (h + 1) * NQ].astype(bf16)),
                "wqT": wqT,
                "wkT": wkT,
                "wvT8": wvT8,
                "pos": pos0 if h == 0 else pos1,
                "smalls": s,
                "onesr": ones_arr,
            }
        )
    return in_maps


def unshard(results):
    out = np.empty((B, C, HW), np.float32)
    for core in range(NCORES):
        b, h = core // 2, core % 2
        out[b][:, h * NQ : (h + 1) * NQ] = results[core]["out"]
    return out.reshape(B, C, HH, WW)


_NC = None


def get_nc():
    global _NC
    if _NC is None:
        _NC = build_bass()
    return _NC


def run_cores(in_maps, **kwargs):
    return run_bass_kernel_spmd(get_nc(), in_maps, core_ids=list(range(NCORES)), **kwargs)


def kernel(**inputs):
    in_maps = make_in_maps(**inputs)
    res = run_cores(in_maps)
    return unshard(res.results)
